# revision 26
# baseline (speedup 1.0000x reference)
"""MoE expert-MLP (SwiGLU) kernel for 8 Trainium2 NeuronCores.

Strategy: expert-parallel, one expert per core. Host-side routing dedups
duplicate (token, expert) slots (K=2 can pick the same expert twice; the
routing weights then just add), which drops the per-expert max count under
2048 and saves a full 128-token tile of padding. Each core runs a dense
[cap, D] SwiGLU MLP in bf16 (tolerance is 2e-2; bf16 lands ~5e-3) which
halves DMA traffic and SBUF footprint vs fp32r at the same PE rate.

Per-core kernel runs block-major over 512-token blocks, with stages
interleaved per block so HBM demand stays flat (~350 GB/s would otherwise
be needed in a front-loaded stage A):
  stage A(b): h^T[h, b] = silu(Wg @ x^T[b]) * (Wu @ x^T[b])
  stage B(b): y[b, d]  = (h^T[b])^T @ Wd^T, row-scaled by routing weight
Wg/Wu/Wd stay SBUF-resident (bf16), x streams one block ahead (the first
block as 16 fine tiles the PE chases chunk-by-chunk at startup, later
blocks as single 2 MB strided DMAs), h lives one block at a time. PSUM is
8 uniform [128, 512] f32 banks: stage A rotates gate/up accumulators by
h-tile parity (4 banks), stage B ping-pongs its 4 output accumulators
between its own bank set and stage A's by t-tile parity, so nothing ever
stalls on a write-after-read hazard.
"""

import sys
import os

sys.path.insert(0, "/opt/trn_rl_repo")

import numpy as np

T, D, H, E, K = 8192, 2048, 1408, 8, 2
P = 128
HT = H // P        # 11 h-tiles
KT = D // P        # 16 d-tiles
G = 512            # PSUM group width (one bank of f32) = token block

_built = {}


def _pass_sizes(cap):
    """Split cap into passes of <=2048 tokens (multiples of 128).
    Expected input fits in a single pass of 2048."""
    sizes = []
    rem = cap
    while rem:
        s = min(rem, 2048)
        if rem - s == 128:  # avoid a tiny trailing pass
            s -= 128
        sizes.append(s)
        rem -= s
    assert all(s % 128 == 0 for s in sizes), sizes
    return sizes


def _groups(tc):
    """Split a pass into 512-wide token blocks (last may be smaller)."""
    return [(o, min(G, tc - o)) for o in range(0, tc, G)]


def _build_nc(cap):
    import concourse.bass as bass  # noqa: F401
    from concourse import bacc
    import concourse.mybir as mybir
    import concourse.tile as tile

    F32 = mybir.dt.float32
    BF16 = mybir.dt.bfloat16
    Silu = mybir.ActivationFunctionType.Silu
    Mult = mybir.AluOpType.mult

    sizes = _pass_sizes(cap)

    nc = bacc.Bacc("TRN2", target_bir_lowering=False, debug=False)
    xT = nc.declare_dram_parameter("xT", [D, cap], BF16, isOutput=False)
    wg = nc.declare_dram_parameter("wg", [HT, P, KT * P], BF16, isOutput=False)
    wu = nc.declare_dram_parameter("wu", [HT, P, KT * P], BF16, isOutput=False)
    wd = nc.declare_dram_parameter("wd", [H, D], BF16, isOutput=False)
    wt = nc.declare_dram_parameter("wt", [cap], F32, isOutput=False)
    out = nc.declare_dram_parameter("out", [cap, D], BF16, isOutput=True)

    xTr = xT.rearrange("(k p) t -> p k t", p=P)  # [128, KT, cap] view
    HW = KT * P // 2

    with tile.TileContext(nc) as tc:
        with (
            tc.tile_pool(name="sbuf", bufs=1) as pool,
            tc.tile_pool(name="psum", bufs=1, space="PSUM") as pp,
        ):
            # ---- startup: keep HBM nearly idle so the first x tiles
            # complete fast. Only h-tiles 0-1 of Wg/Wu load up front; the
            # rest of the weight stream is throttled behind stage-A silu
            # ops (one h-tile's weights per silu), so x-completion never
            # queues behind a saturated weight stream.
            def load_w_half(mat, ht, hf, tagp, eng):
                w1 = pool.tile([P, HW], BF16, tag=f"{tagp}{ht}h{hf}", bufs=1)
                eng.dma_start(w1[:], mat[ht, :, hf * HW : (hf + 1) * HW])
                return w1

            wg_h = [[None, None] for _ in range(HT)]
            wu_h = [[None, None] for _ in range(HT)]

            def load_w_ht(ht, eng):
                for hf in range(2):
                    wg_h[ht][hf] = load_w_half(wg, ht, hf, "wg", eng)
                for hf in range(2):
                    wu_h[ht][hf] = load_w_half(wu, ht, hf, "wu", eng)

            wd_ts = [None] * HT

            def load_wd(ht, eng):
                wdc = pool.tile([P, D], BF16, tag=f"wd{ht}", bufs=1,
                                name=f"wdc{ht}")
                eng.dma_start(wdc[:], wd[ht * P : (ht + 1) * P, :])
                wd_ts[ht] = wdc

            # first block's x: 16 fine [128, 512] tiles. The first two even
            # chunks ride the near-empty sync HWDGE queue (fastest
            # first-byte), the rest alternate gpsimd/scalar.
            g0 = _groups(sizes[0])[0][1]
            xf_ts = [None] * KT

            # zeros tile for the PE warm-up: first thing on gpsimd so the
            # warm-up matmuls can start right after the preamble
            zt = pool.tile([P, G], BF16, tag="zwarm", bufs=1)
            nc.gpsimd.memset(zt[:], 0.0)

            def load_xf(d, eng):
                x1 = pool.tile([P, G], BF16, tag=f"xf{d}", bufs=1,
                               name=f"xf{d}")
                eng.dma_start(x1[:, :g0], xT[d * P : (d + 1) * P, 0:g0])
                xf_ts[d] = x1

            load_xf(0, nc.sync)
            for hf in range(2):
                wg_h[0][hf] = load_w_half(wg, 0, hf, "wg", nc.sync)
            load_xf(2, nc.sync)
            for d in range(KT):
                if xf_ts[d] is None:
                    load_xf(d, nc.gpsimd if d % 2 == 0 else nc.scalar)
            for hf in range(2):
                wu_h[0][hf] = load_w_half(wu, 0, hf, "wu", nc.sync)
            load_w_ht(1, nc.sync)

            wt_t = pool.tile([P, cap // P], F32, tag="wt", bufs=1)
            nc.sync.dma_start(wt_t[:], wt.rearrange("(n p) -> p n", p=P))

            # Warm-up matmuls on the zeros tile: dependency-free, they fill
            # the dead window before the first x chunk lands and carry the
            # PE through its DVFS ramp. Dead stores into the stage-B banks,
            # which stage B resets with start=True before first use.
            for wi in range(8):
                pw = pp.tile([P, G], F32, tag=f"y{wi % 4}", name="warm")
                nc.tensor.matmul(pw[:], zt[:, 0:P], zt[:],
                                 start=True, stop=True)

            first_block = True
            t0 = 0
            for pi, TC in enumerate(sizes):
                grps = _groups(TC)
                xblk = [None] * len(grps)

                def load_big(off, g, eng):
                    x1 = pool.tile([P, KT, G], BF16, tag="xb", bufs=2,
                                   name="xb")
                    eng.dma_start(
                        x1[:, :, :g], xTr[:, :, t0 + off : t0 + off + g]
                    )
                    return lambda d: x1[:, d, :]

                for bi, (off, g) in enumerate(grps):
                    if first_block:
                        xblk[0] = (lambda d: xf_ts[d])
                    if xblk[bi] is None:
                        xblk[bi] = load_big(off, g, nc.gpsimd)
                    xs = xblk[bi]

                    # h^T for this block only
                    h_t = pool.tile([P, HT, G], BF16, tag="ht", bufs=1)

                    # ---- stage A(b) ----
                    for ht in range(HT):
                        psg = pp.tile([P, G], F32, tag=f"g{ht % 2}",
                                      name="psg")
                        psu = pp.tile([P, G], F32, tag=f"u{ht % 2}",
                                      name="psu")

                        def gate(d):
                            lhs = wg_h[ht][d // 8][:, (d % 8) * P : (d % 8 + 1) * P]
                            nc.tensor.matmul(
                                psg[:, :g], lhs, xs(d)[:, :g],
                                start=(d == 0), stop=(d == KT - 1),
                            )

                        def up(d):
                            lhs = wu_h[ht][d // 8][:, (d % 8) * P : (d % 8 + 1) * P]
                            nc.tensor.matmul(
                                psu[:, :g], lhs, xs(d)[:, :g],
                                start=(d == 0), stop=(d == KT - 1),
                            )

                        if first_block and ht == 0:
                            # consume each x chunk twice back-to-back so
                            # the PE paces with the incoming x stream
                            for d in range(KT):
                                gate(d)
                                up(d)
                            st = pool.tile([P, G], F32, tag="silu", bufs=2,
                                           name="st")
                            nc.scalar.activation(st[:, :g], psg[:, :g], Silu)
                        else:
                            for d in range(KT):
                                gate(d)
                            st = pool.tile([P, G], F32, tag="silu", bufs=2,
                                           name="st")
                            nc.scalar.activation(st[:, :g], psg[:, :g], Silu)
                            for d in range(KT):
                                up(d)
                        nc.vector.tensor_tensor(
                            h_t[:, ht, :g], st[:, :g], psu[:, :g], op=Mult,
                        )
                        # Throttled streams, queued on scalar BEHIND the
                        # silu just emitted: the sequencer only issues them
                        # after that silu runs, so transfers start after
                        # the startup x stream is done and trickle in at
                        # compute pace. Covers Wg/Wu h-tiles 2-10, all of
                        # Wd, and the block-1/2 x prefetches.
                        if first_block:
                            if ht + 2 < HT:
                                load_w_ht(ht + 2, nc.scalar)
                            if 5 <= ht <= 9:
                                for j in ((ht - 5) * 2, (ht - 5) * 2 + 1):
                                    load_wd(j, nc.scalar)
                            if ht == 10:
                                load_wd(10, nc.scalar)
                            if ht == 2 and len(grps) > 1 and xblk[1] is None:
                                xblk[1] = load_big(*grps[1], nc.scalar)
                            if ht == 4 and len(grps) > 2 and xblk[2] is None:
                                xblk[2] = load_big(*grps[2], nc.scalar)
                    nb = bi + 2
                    if nb < len(grps) and xblk[nb] is None:
                        xblk[nb] = load_big(*grps[nb], nc.gpsimd)
                    first_block = False

                    # ---- stage B(b): 4 t-tiles of 128 tokens ----
                    for ts_ in range(g // P):
                        last_tile = (pi == len(sizes) - 1
                                     and bi == len(grps) - 1
                                     and ts_ == g // P - 1)
                        ptags = (["y0", "y1", "y2", "y3"] if ts_ % 2 == 0
                                 else ["g0", "g1", "u0", "u1"])
                        psy = [pp.tile([P, G], F32, tag=ptags[i],
                                       name=f"psy{i}") for i in range(4)]
                        y_t = pool.tile([P, D], BF16, tag="yout", bufs=2,
                                        name="y_t")
                        col = (t0 + off) // P + ts_
                        for dc in range(4):
                            for ht in range(HT):
                                nc.tensor.matmul(
                                    psy[dc][:],
                                    h_t[:, ht, ts_ * P : (ts_ + 1) * P],
                                    wd_ts[ht][:, dc * G : (dc + 1) * G],
                                    start=(ht == 0),
                                    stop=(ht == HT - 1),
                                )
                            nc.vector.tensor_scalar_mul(
                                y_t[:, dc * G : (dc + 1) * G],
                                psy[dc][:],
                                wt_t[:, col : col + 1],
                            )
                            if last_tile:
                                # drain the final tile per 512-group so the
                                # kernel doesn't wait on one big store
                                nc.sync.dma_start(
                                    out[
                                        t0 + off + ts_ * P :
                                        t0 + off + (ts_ + 1) * P,
                                        dc * G : (dc + 1) * G,
                                    ],
                                    y_t[:, dc * G : (dc + 1) * G],
                                )
                        if not last_tile:
                            nc.sync.dma_start(
                                out[t0 + off + ts_ * P : t0 + off + (ts_ + 1) * P, :],
                                y_t[:],
                            )
                t0 += TC

    nc.finalize()
    return nc


def _get_nc(cap):
    if cap not in _built:
        _built[cap] = _build_nc(cap)
    return _built[cap]


def kernel(x, weights, Wg, Wu, Wd, indices, seq_len=None, **_unused):
    from concourse.bass_utils import run_bass_kernel_spmd
    import ml_dtypes

    bf16 = ml_dtypes.bfloat16

    x = np.asarray(x, dtype=np.float32)
    weights = np.asarray(weights, dtype=np.float32)
    Wg = np.asarray(Wg, dtype=np.float32)
    Wu = np.asarray(Wu, dtype=np.float32)
    Wd = np.asarray(Wd, dtype=np.float32)
    indices = np.asarray(indices).astype(np.int64)

    t, d = x.shape
    e, h, _ = Wg.shape
    k = indices.shape[1]

    # ---- host-side routing (dispatch), merging duplicate (token, expert)
    # slots so their routing weights add and each pair is computed once ----
    flat_e = indices.reshape(-1)
    flat_t = np.repeat(np.arange(t, dtype=np.int64), k)
    flat_w = weights.reshape(-1)
    key = flat_e * t + flat_t                       # sort by (expert, token)
    order = np.argsort(key, kind="stable")
    ks = key[order]
    ws = flat_w[order]
    uniq = np.empty(len(ks), dtype=bool)
    uniq[0] = True
    uniq[1:] = ks[1:] != ks[:-1]
    seg = np.cumsum(uniq) - 1
    w_u = np.bincount(seg, weights=ws).astype(np.float32)  # summed weights
    k_u = ks[uniq]
    e_u = k_u // t
    tok_u = k_u % t
    counts = np.bincount(e_u, minlength=e)
    starts = np.zeros(e + 1, dtype=np.int64)
    starts[1:] = np.cumsum(counts)
    cap = int(-(-max(int(counts.max()), 512) // P) * P)

    in_maps = []
    for ei in range(e):
        n = int(counts[ei])
        toks = tok_u[starts[ei] : starts[ei] + n]
        xe = np.zeros((cap, d), dtype=np.float32)
        xe[:n] = x[toks]
        wvec = np.zeros(cap, dtype=np.float32)
        wvec[:n] = w_u[starts[ei] : starts[ei] + n]
        # pack Wg/Wu so each h-tile block is one contiguous [128, 2048] DMA:
        # block[ht][p][k*128+hh] = Wg[e].T[k*128+p, ht*128+hh]
        WgT = Wg[ei].T  # [D, H]
        WuT = Wu[ei].T
        wg_lin = np.ascontiguousarray(
            WgT.reshape(KT, P, HT, P).transpose(2, 1, 0, 3).reshape(HT, P, KT * P)
        ).astype(bf16)
        wu_lin = np.ascontiguousarray(
            WuT.reshape(KT, P, HT, P).transpose(2, 1, 0, 3).reshape(HT, P, KT * P)
        ).astype(bf16)
        wdT = np.ascontiguousarray(Wd[ei].T).astype(bf16)  # [H, D]
        in_maps.append(
            {
                "xT": np.ascontiguousarray(xe.T).astype(bf16),
                "wg": wg_lin,
                "wu": wu_lin,
                "wd": wdT,
                "wt": wvec,
            }
        )

    nc = _get_nc(cap)
    trace = bool(int(os.environ.get("KERNEL_TRACE", "0")))
    res = run_bass_kernel_spmd(
        nc, in_maps, core_ids=list(range(e)), trace=trace
    )
    if trace:
        kernel.last_exec_time_ns = res.exec_time_ns
        kernel.last_results = res

    # ---- host-side combine ----
    allres = np.concatenate(
        [np.asarray(res.results[ei]["out"][: counts[ei]]) for ei in range(e)],
        axis=0,
    ).astype(np.float32)
    allres = np.vstack([allres, np.zeros((1, d), np.float32)])
    # map each original (token, k) slot to its deduped row; the second slot
    # of an intra-token duplicate points at the trailing zeros row
    pos = np.searchsorted(k_u, key).reshape(t, k)
    dup = indices[:, 0] == indices[:, 1]
    pos[dup, 1] = len(k_u)
    y = allres[pos[:, 0]] + allres[pos[:, 1]]
    return y


# revision 27
# speedup vs baseline: 1.0052x; 1.0052x over previous
"""MoE expert-MLP (SwiGLU) kernel for 8 Trainium2 NeuronCores.

Strategy: expert-parallel, one expert per core. Host-side routing dedups
duplicate (token, expert) slots (K=2 can pick the same expert twice; the
routing weights then just add), which drops the per-expert max count under
2048 and saves a full 128-token tile of padding. Each core runs a dense
[cap, D] SwiGLU MLP in bf16 (tolerance is 2e-2; bf16 lands ~5e-3) which
halves DMA traffic and SBUF footprint vs fp32r at the same PE rate.

Per-core kernel runs block-major over 512-token blocks, with stages
interleaved per block so HBM demand stays flat (~350 GB/s would otherwise
be needed in a front-loaded stage A):
  stage A(b): h^T[h, b] = silu(Wg @ x^T[b]) * (Wu @ x^T[b])
  stage B(b): y[b, d]  = (h^T[b])^T @ Wd^T, row-scaled by routing weight
Wg/Wu/Wd stay SBUF-resident (bf16), x streams one block ahead (the first
block as 16 fine tiles the PE chases chunk-by-chunk at startup, later
blocks as single 2 MB strided DMAs), h lives one block at a time. PSUM is
8 uniform [128, 512] f32 banks: stage A rotates gate/up accumulators by
h-tile parity (4 banks), stage B ping-pongs its 4 output accumulators
between its own bank set and stage A's by t-tile parity, so nothing ever
stalls on a write-after-read hazard.
"""

import sys
import os

sys.path.insert(0, "/opt/trn_rl_repo")

import numpy as np

T, D, H, E, K = 8192, 2048, 1408, 8, 2
P = 128
HT = H // P        # 11 h-tiles
KT = D // P        # 16 d-tiles
G = 512            # PSUM group width (one bank of f32) = token block

_built = {}


def _pass_sizes(cap):
    """Split cap into passes of <=2048 tokens (multiples of 128).
    Expected input fits in a single pass of 2048."""
    sizes = []
    rem = cap
    while rem:
        s = min(rem, 2048)
        if rem - s == 128:  # avoid a tiny trailing pass
            s -= 128
        sizes.append(s)
        rem -= s
    assert all(s % 128 == 0 for s in sizes), sizes
    return sizes


def _groups(tc):
    """Split a pass into 512-wide token blocks (last may be smaller)."""
    return [(o, min(G, tc - o)) for o in range(0, tc, G)]


def _build_nc(cap):
    import concourse.bass as bass  # noqa: F401
    from concourse import bacc
    import concourse.mybir as mybir
    import concourse.tile as tile

    F32 = mybir.dt.float32
    BF16 = mybir.dt.bfloat16
    Silu = mybir.ActivationFunctionType.Silu
    Mult = mybir.AluOpType.mult

    sizes = _pass_sizes(cap)

    nc = bacc.Bacc("TRN2", target_bir_lowering=False, debug=False)
    xT = nc.declare_dram_parameter("xT", [D, cap], BF16, isOutput=False)
    wg = nc.declare_dram_parameter("wg", [HT, P, KT * P], BF16, isOutput=False)
    wu = nc.declare_dram_parameter("wu", [HT, P, KT * P], BF16, isOutput=False)
    wd = nc.declare_dram_parameter("wd", [H, D], BF16, isOutput=False)
    wt = nc.declare_dram_parameter("wt", [cap], F32, isOutput=False)
    out = nc.declare_dram_parameter("out", [cap, D], BF16, isOutput=True)

    xTr = xT.rearrange("(k p) t -> p k t", p=P)  # [128, KT, cap] view
    HW = KT * P // 2

    with tile.TileContext(nc) as tc:
        with (
            tc.tile_pool(name="sbuf", bufs=1) as pool,
            tc.tile_pool(name="psum", bufs=1, space="PSUM") as pp,
        ):
            # ---- startup: keep HBM nearly idle so the first x tiles
            # complete fast. Only h-tiles 0-1 of Wg/Wu load up front; the
            # rest of the weight stream is throttled behind stage-A silu
            # ops (one h-tile's weights per silu), so x-completion never
            # queues behind a saturated weight stream.
            def load_w_half(mat, ht, hf, tagp, eng):
                w1 = pool.tile([P, HW], BF16, tag=f"{tagp}{ht}h{hf}", bufs=1)
                eng.dma_start(w1[:], mat[ht, :, hf * HW : (hf + 1) * HW])
                return w1

            wg_h = [[None, None] for _ in range(HT)]
            wu_h = [[None, None] for _ in range(HT)]

            def load_w_ht(ht, eng):
                for hf in range(2):
                    wg_h[ht][hf] = load_w_half(wg, ht, hf, "wg", eng)
                for hf in range(2):
                    wu_h[ht][hf] = load_w_half(wu, ht, hf, "wu", eng)

            wd_ts = [None] * HT

            def load_wd(ht, eng):
                wdc = pool.tile([P, D], BF16, tag=f"wd{ht}", bufs=1,
                                name=f"wdc{ht}")
                eng.dma_start(wdc[:], wd[ht * P : (ht + 1) * P, :])
                wd_ts[ht] = wdc

            # first block's x: 16 fine [128, 512] tiles on gpsimd/scalar
            g0 = _groups(sizes[0])[0][1]
            xf_ts = []
            for d in range(KT):
                x1 = pool.tile([P, G], BF16, tag=f"xf{d}", bufs=1,
                               name=f"xf{d}")
                eng = nc.gpsimd if d % 2 == 0 else nc.scalar
                eng.dma_start(x1[:, :g0], xT[d * P : (d + 1) * P, 0:g0])
                xf_ts.append(x1)

            load_w_ht(0, nc.sync)
            load_w_ht(1, nc.sync)

            wt_t = pool.tile([P, cap // P], F32, tag="wt", bufs=1)
            nc.sync.dma_start(wt_t[:], wt.rearrange("(n p) -> p n", p=P))

            first_block = True
            t0 = 0
            for pi, TC in enumerate(sizes):
                grps = _groups(TC)
                xblk = [None] * len(grps)

                def load_big(off, g, eng):
                    x1 = pool.tile([P, KT, G], BF16, tag="xb", bufs=2,
                                   name="xb")
                    eng.dma_start(
                        x1[:, :, :g], xTr[:, :, t0 + off : t0 + off + g]
                    )
                    return lambda d: x1[:, d, :]

                for bi, (off, g) in enumerate(grps):
                    if first_block:
                        xblk[0] = (lambda d: xf_ts[d])
                    if xblk[bi] is None:
                        xblk[bi] = load_big(off, g, nc.gpsimd)
                    xs = xblk[bi]

                    # h^T for this block only
                    h_t = pool.tile([P, HT, G], BF16, tag="ht", bufs=1)

                    # ---- stage A(b) ----
                    for ht in range(HT):
                        psg = pp.tile([P, G], F32, tag=f"g{ht % 2}",
                                      name="psg")
                        psu = pp.tile([P, G], F32, tag=f"u{ht % 2}",
                                      name="psu")

                        def gate(d):
                            lhs = wg_h[ht][d // 8][:, (d % 8) * P : (d % 8 + 1) * P]
                            nc.tensor.matmul(
                                psg[:, :g], lhs, xs(d)[:, :g],
                                start=(d == 0), stop=(d == KT - 1),
                            )

                        def up(d):
                            lhs = wu_h[ht][d // 8][:, (d % 8) * P : (d % 8 + 1) * P]
                            nc.tensor.matmul(
                                psu[:, :g], lhs, xs(d)[:, :g],
                                start=(d == 0), stop=(d == KT - 1),
                            )

                        if first_block and ht == 0:
                            # consume each x chunk twice back-to-back so
                            # the PE paces with the incoming x stream
                            for d in range(KT):
                                gate(d)
                                up(d)
                            st = pool.tile([P, G], F32, tag="silu", bufs=2,
                                           name="st")
                            nc.scalar.activation(st[:, :g], psg[:, :g], Silu)
                        else:
                            for d in range(KT):
                                gate(d)
                            st = pool.tile([P, G], F32, tag="silu", bufs=2,
                                           name="st")
                            nc.scalar.activation(st[:, :g], psg[:, :g], Silu)
                            for d in range(KT):
                                up(d)
                        nc.vector.tensor_tensor(
                            h_t[:, ht, :g], st[:, :g], psu[:, :g], op=Mult,
                        )
                        # Throttled streams, queued on scalar BEHIND the
                        # silu just emitted: the sequencer only issues them
                        # after that silu runs, so transfers start after
                        # the startup x stream is done and trickle in at
                        # compute pace. Covers Wg/Wu h-tiles 2-10, all of
                        # Wd, and the block-1/2 x prefetches.
                        if first_block:
                            if ht + 2 < HT:
                                load_w_ht(ht + 2, nc.scalar)
                            if 5 <= ht <= 9:
                                for j in ((ht - 5) * 2, (ht - 5) * 2 + 1):
                                    load_wd(j, nc.scalar)
                            if ht == 10:
                                load_wd(10, nc.scalar)
                            if ht == 2 and len(grps) > 1 and xblk[1] is None:
                                xblk[1] = load_big(*grps[1], nc.scalar)
                            if ht == 4 and len(grps) > 2 and xblk[2] is None:
                                xblk[2] = load_big(*grps[2], nc.scalar)
                    nb = bi + 2
                    if nb < len(grps) and xblk[nb] is None:
                        xblk[nb] = load_big(*grps[nb], nc.gpsimd)
                    first_block = False

                    # ---- stage B(b): 4 t-tiles of 128 tokens ----
                    for ts_ in range(g // P):
                        last_tile = (pi == len(sizes) - 1
                                     and bi == len(grps) - 1
                                     and ts_ == g // P - 1)
                        ptags = (["y0", "y1", "y2", "y3"] if ts_ % 2 == 0
                                 else ["g0", "g1", "u0", "u1"])
                        psy = [pp.tile([P, G], F32, tag=ptags[i],
                                       name=f"psy{i}") for i in range(4)]
                        y_t = pool.tile([P, D], BF16, tag="yout", bufs=2,
                                        name="y_t")
                        col = (t0 + off) // P + ts_
                        for dc in range(4):
                            for ht in range(HT):
                                nc.tensor.matmul(
                                    psy[dc][:],
                                    h_t[:, ht, ts_ * P : (ts_ + 1) * P],
                                    wd_ts[ht][:, dc * G : (dc + 1) * G],
                                    start=(ht == 0),
                                    stop=(ht == HT - 1),
                                )
                            nc.vector.tensor_scalar_mul(
                                y_t[:, dc * G : (dc + 1) * G],
                                psy[dc][:],
                                wt_t[:, col : col + 1],
                            )
                            if last_tile:
                                # drain the final tile per 512-group so the
                                # kernel doesn't wait on one big store
                                nc.sync.dma_start(
                                    out[
                                        t0 + off + ts_ * P :
                                        t0 + off + (ts_ + 1) * P,
                                        dc * G : (dc + 1) * G,
                                    ],
                                    y_t[:, dc * G : (dc + 1) * G],
                                )
                        if not last_tile:
                            nc.sync.dma_start(
                                out[t0 + off + ts_ * P : t0 + off + (ts_ + 1) * P, :],
                                y_t[:],
                            )
                t0 += TC

    nc.finalize()
    return nc


def _get_nc(cap):
    if cap not in _built:
        _built[cap] = _build_nc(cap)
    return _built[cap]


def kernel(x, weights, Wg, Wu, Wd, indices, seq_len=None, **_unused):
    from concourse.bass_utils import run_bass_kernel_spmd
    import ml_dtypes

    bf16 = ml_dtypes.bfloat16

    x = np.asarray(x, dtype=np.float32)
    weights = np.asarray(weights, dtype=np.float32)
    Wg = np.asarray(Wg, dtype=np.float32)
    Wu = np.asarray(Wu, dtype=np.float32)
    Wd = np.asarray(Wd, dtype=np.float32)
    indices = np.asarray(indices).astype(np.int64)

    t, d = x.shape
    e, h, _ = Wg.shape
    k = indices.shape[1]

    # ---- host-side routing (dispatch), merging duplicate (token, expert)
    # slots so their routing weights add and each pair is computed once ----
    flat_e = indices.reshape(-1)
    flat_t = np.repeat(np.arange(t, dtype=np.int64), k)
    flat_w = weights.reshape(-1)
    key = flat_e * t + flat_t                       # sort by (expert, token)
    order = np.argsort(key, kind="stable")
    ks = key[order]
    ws = flat_w[order]
    uniq = np.empty(len(ks), dtype=bool)
    uniq[0] = True
    uniq[1:] = ks[1:] != ks[:-1]
    seg = np.cumsum(uniq) - 1
    w_u = np.bincount(seg, weights=ws).astype(np.float32)  # summed weights
    k_u = ks[uniq]
    e_u = k_u // t
    tok_u = k_u % t
    counts = np.bincount(e_u, minlength=e)
    starts = np.zeros(e + 1, dtype=np.int64)
    starts[1:] = np.cumsum(counts)
    cap = int(-(-max(int(counts.max()), 512) // P) * P)

    in_maps = []
    for ei in range(e):
        n = int(counts[ei])
        toks = tok_u[starts[ei] : starts[ei] + n]
        xe = np.zeros((cap, d), dtype=np.float32)
        xe[:n] = x[toks]
        wvec = np.zeros(cap, dtype=np.float32)
        wvec[:n] = w_u[starts[ei] : starts[ei] + n]
        # pack Wg/Wu so each h-tile block is one contiguous [128, 2048] DMA:
        # block[ht][p][k*128+hh] = Wg[e].T[k*128+p, ht*128+hh]
        WgT = Wg[ei].T  # [D, H]
        WuT = Wu[ei].T
        wg_lin = np.ascontiguousarray(
            WgT.reshape(KT, P, HT, P).transpose(2, 1, 0, 3).reshape(HT, P, KT * P)
        ).astype(bf16)
        wu_lin = np.ascontiguousarray(
            WuT.reshape(KT, P, HT, P).transpose(2, 1, 0, 3).reshape(HT, P, KT * P)
        ).astype(bf16)
        wdT = np.ascontiguousarray(Wd[ei].T).astype(bf16)  # [H, D]
        in_maps.append(
            {
                "xT": np.ascontiguousarray(xe.T).astype(bf16),
                "wg": wg_lin,
                "wu": wu_lin,
                "wd": wdT,
                "wt": wvec,
            }
        )

    nc = _get_nc(cap)
    trace = bool(int(os.environ.get("KERNEL_TRACE", "0")))
    res = run_bass_kernel_spmd(
        nc, in_maps, core_ids=list(range(e)), trace=trace
    )
    if trace:
        kernel.last_exec_time_ns = res.exec_time_ns
        kernel.last_results = res

    # ---- host-side combine ----
    allres = np.concatenate(
        [np.asarray(res.results[ei]["out"][: counts[ei]]) for ei in range(e)],
        axis=0,
    ).astype(np.float32)
    allres = np.vstack([allres, np.zeros((1, d), np.float32)])
    # map each original (token, k) slot to its deduped row; the second slot
    # of an intra-token duplicate points at the trailing zeros row
    pos = np.searchsorted(k_u, key).reshape(t, k)
    dup = indices[:, 0] == indices[:, 1]
    pos[dup, 1] = len(k_u)
    y = allres[pos[:, 0]] + allres[pos[:, 1]]
    return y
